# revision 12
# baseline (speedup 1.0000x reference)
"""Two-layer single-head GAT (PyG GATConv semantics) on 8 Trainium2 NeuronCores.

Strategy (dst-sharded edge parallelism):
  - Nodes split into 8 equal contiguous ranges; core c owns nodes
    [c*nps, (c+1)*nps) and every edge whose dst lands there, so the
    per-dst softmax and aggregation are core-local.
  - Per layer, each core computes the augmented bf16 feature table for
    its node slice (feats | a_src | 1.0) and the slices are AllGathered
    so every core holds the full [N, TW] table in its HBM.
  - Edges are grouped by (src-bucket, dst-block); dma_gather (int16
    indices over <=25000-row bucket views) pulls src rows for up to
    3072 edges per call.  A one-hot [128-edge x 128-dst] matrix built
    on DVE (is_equal vs iota) broadcasts the per-dst attention scalar
    to edges and, as the matmul stationary operand, performs the
    segment aggregation into PSUM; the table's ones column makes the
    same matmul produce the softmax denominator.
  - Bucket passes accumulate via per-block SBUF accumulators
    (PSUM -> SBUF ScalarE copy, re-injected with an identity matmul).
  - The final softmax over axis 0 uses one [h2]-float AllReduce.
"""

import numpy as np

import sys

_BASS_ROOT = "/opt/trn_rl_repo"
if _BASS_ROOT not in sys.path:
    sys.path.insert(0, _BASS_ROOT)

import ml_dtypes

import concourse.bass as bass
import concourse.bacc as bacc
import concourse.tile as tile
from concourse import mybir
from concourse.bass_utils import run_bass_kernel_spmd
from concourse.masks import make_identity

import bass_rust
from concourse.tile import ScopedClock

F32 = mybir.dt.float32
BF16 = mybir.dt.bfloat16
I16 = mybir.dt.int16
NEG_SLOPE_GAT = 0.2
NEG_SLOPE_ACT = 0.01
WCOLS = 24  # max gather-window width in 128-edge columns
P = 128


class SplitDrainTileContext(tile.TileContext):
    """Workaround for walrus 'Too many sync wait commands' on the final
    drain: spread the global-clock waits across one-wait NOPs."""

    def _drain_and_barrier(self, tick_clock, wait_clock):
        nc = self.nc
        drain_inst = nc.sync.drain()
        wait_clock.add_sem_waits(
            drain_inst.ins, ScopedClock({None: tick_clock.global_clock})
        )
        si = drain_inst.ins.sync_info
        waits = list(si.on_wait) if (si is not None and si.on_wait) else []
        if len(waits) > 1:
            si.on_wait = [waits[0]]
            for w in waits[1:]:
                nop = nc.sync.nop()
                nop.ins.sync_info = bass_rust.SyncInfo(on_wait=[w], on_update=[])
        nc.all_engine_barrier()
        assert self.sems is not None
        popped = nc._tile_sem_poison_stack.pop()
        assert popped is self._sem_poison
        nc.clear_and_free_semaphores(list(self.sems.allocated().values()))
        nc.all_engine_barrier()




_WSPLIT_CTR = [0]


def _split_waits(nc, maxw=1):
    """This walrus build rejects instructions carrying more than ~2 sync
    waits; hoist the excess onto same-engine NOPs placed just before."""
    for bb in nc.main_func.blocks:
        out = []
        for ins in bb.instructions:
            si = ins.sync_info
            waits = list(si.on_wait) if (si is not None and si.on_wait) else []
            if len(waits) > maxw:
                for i in range(maxw, len(waits), maxw):
                    _WSPLIT_CTR[0] += 1
                    nop = mybir.InstNoOp(name=f"wsplit_{_WSPLIT_CTR[0]}",
                                         ins=[], outs=[])
                    nop.engine = ins.engine
                    nop.sync_info = bass_rust.SyncInfo(
                        on_wait=waits[i:i + maxw], on_update=[])
                    out.append(nop)
                si.on_wait = waits[:maxw]
            out.append(ins)
        bb.instructions[:] = out


# ----------------------------------------------------------------------------
# Host-side schedule construction
# ----------------------------------------------------------------------------

def _build_schedule(src, dst, N, n_cores, BK):
    """Group edges by (core=dst-range, bucket=src-range, block=dst//128).
    Each (bucket, block) run is padded to whole 128-edge columns with run
    sizes maxed over cores so all cores share one static program."""
    nps = N // n_cores
    assert nps * n_cores == N
    NBLK = (nps + P - 1) // P
    nbuckets = (N + BK - 1) // BK
    core = dst // nps
    dstl = dst - core * nps
    block = dstl // P
    bucket = src // BK

    counts = np.zeros((n_cores, nbuckets, NBLK), dtype=np.int64)
    np.add.at(counts, (core, bucket, block), 1)
    cmax = counts.max(axis=0)
    cols = (cmax + P - 1) // P  # [nbuckets, NBLK]

    # column layout: bucket-major, block order inside each bucket
    col_off = np.zeros((nbuckets, NBLK), dtype=np.int64)
    C_k = cols.sum(axis=1)
    k_off = np.concatenate([[0], np.cumsum(C_k)])[:-1]
    running = k_off.copy()
    for b in range(NBLK):
        col_off[:, b] = running
        running = running + cols[:, b]
    C_tot = int(cols.sum())

    first_pass = np.zeros(NBLK, dtype=np.int64)
    last_pass = np.zeros(NBLK, dtype=np.int64)
    for b in range(NBLK):
        ks = np.nonzero(cols[:, b] > 0)[0]
        assert len(ks) > 0, f"block {b} has no edges (impossible w/ self-loops)"
        first_pass[b], last_pass[b] = ks[0], ks[-1]

    # windows: whole blocks packed up to WCOLS columns; oversized blocks get
    # solo split windows (their PSUM tile safely spans the splits).
    windows = []
    for k in range(nbuckets):
        b = 0
        while b < NBLK:
            ck = int(cols[k, b])
            if ck == 0:
                b += 1
                continue
            if ck > WCOLS:
                c0b = int(col_off[k, b])
                for w0 in range(0, ck, WCOLS):
                    cw = min(WCOLS, ck - w0)
                    windows.append(dict(
                        k=k, c0=c0b + w0, cw=cw,
                        segs=[dict(b=b, lo=0, hi=cw, first=(w0 == 0),
                                   last=(w0 + cw == ck))]))
                b += 1
                continue
            group = []
            tot = 0
            c0 = int(col_off[k, b])
            while b < NBLK and tot + int(cols[k, b]) <= WCOLS:
                if cols[k, b] > 0:
                    group.append(b)
                    tot += int(cols[k, b])
                b += 1
            segs = []
            for gb in group:
                lo = int(col_off[k, gb]) - c0
                segs.append(dict(b=gb, lo=lo, hi=lo + int(cols[k, gb]),
                                 first=True, last=True))
            windows.append(dict(k=k, c0=c0, cw=tot, segs=segs))

    bucket_rows = [min(BK, N - k * BK) for k in range(nbuckets)]
    return dict(
        nps=nps, NBLK=NBLK, nbuckets=nbuckets, BK=BK, bucket_rows=bucket_rows,
        cols=cols, col_off=col_off, C_tot=C_tot,
        first_pass=first_pass, last_pass=last_pass, windows=windows,
        n_cores=n_cores, N=N,
    )


def _build_edge_arrays(src, dst, sched):
    """Per-core gidx (wrapped int16 for dma_gather) and dstrel (bf16)."""
    n_cores, N, nps = sched["n_cores"], sched["N"], sched["nps"]
    NBLK, nbuckets, BK = sched["NBLK"], sched["nbuckets"], sched["BK"]
    col_off, C_tot = sched["col_off"], sched["C_tot"]

    core = dst // nps
    gidx_all, dstrel_all = [], []
    for c in range(n_cores):
        m = core == c
        s_c, d_c = src[m], dst[m] - c * nps
        blk = d_c // P
        bkt = s_c // BK
        order = np.lexsort((s_c, blk, bkt))
        s_c, d_c, blk, bkt = s_c[order], d_c[order], blk[order], bkt[order]

        slot_idx = np.zeros(C_tot * P, dtype=np.int32)
        slot_rel = np.full(C_tot * P, 255.0, dtype=np.float32)
        if len(s_c):
            newgrp = np.ones(len(s_c), dtype=bool)
            newgrp[1:] = (bkt[1:] != bkt[:-1]) | (blk[1:] != blk[:-1])
            starts = np.nonzero(newgrp)[0]
            reps = np.diff(np.concatenate([starts, [len(s_c)]]))
            within = np.arange(len(s_c)) - np.repeat(starts, reps)
            slot = col_off[bkt, blk] * P + within
            slot_idx[slot] = (s_c - bkt * BK).astype(np.int32)
            slot_rel[slot] = (d_c - blk * P).astype(np.float32)

        # flat slot s (within the global column space) = col*128 + p
        dstrel = slot_rel.reshape(C_tot, P).T.astype(ml_dtypes.bfloat16)

        gidx = np.zeros((P, C_tot * 8), dtype=np.int16)
        for w in sched["windows"]:
            c0, cw = w["c0"], w["cw"]
            seg = slot_idx[c0 * P:(c0 + cw) * P].astype(np.int16)
            wrapped = seg.reshape(cw * 8, 16).T  # [16, cw*8]
            gidx[:, c0 * 8:(c0 + cw) * 8] = np.tile(wrapped, (8, 1))
        gidx_all.append(np.ascontiguousarray(gidx))
        dstrel_all.append(np.ascontiguousarray(dstrel))
    return gidx_all, dstrel_all


# ----------------------------------------------------------------------------
# Device program
# ----------------------------------------------------------------------------

def _build_program(sched, d_in, h1, h2):
    nps, NBLK = sched["nps"], sched["NBLK"]
    nbuckets, BK = sched["nbuckets"], sched["BK"]
    C_tot, N = sched["C_tot"], sched["N"]
    n_cores = sched["n_cores"]
    core_ids = list(range(n_cores))
    assert d_in == 128 and h1 == 128

    TW1, TW2 = 256, 128          # bf16 table row widths (512B / 256B)
    NW1, NW2 = h1 + 2, h2 + 2    # matmul widths: feats + asrc-junk-col + ones
    AUG1, AUG2 = d_in + 4, h1 + 8
    NWMAX = NW1

    nc = bacc.Bacc(None)
    x_in = nc.dram_tensor("x_slice", [nps, d_in], F32, kind="ExternalInput")
    W1a = nc.dram_tensor("W1_aug", [d_in, AUG1], F32, kind="ExternalInput")
    W2a = nc.dram_tensor("W2_aug", [h1, AUG2], F32, kind="ExternalInput")
    b1_in = nc.dram_tensor("b1r", [1, h1], F32, kind="ExternalInput")
    b2_in = nc.dram_tensor("b2r", [1, h2], F32, kind="ExternalInput")
    gidx_in = nc.dram_tensor("gidx", [P, C_tot * 8], I16, kind="ExternalInput")
    dstrel_in = nc.dram_tensor("dstrel", [P, C_tot], BF16, kind="ExternalInput")
    out_ext = nc.dram_tensor("out_slice", [nps, h2], F32, kind="ExternalOutput")

    tbl1_slice = nc.dram_tensor("tbl1_slice", [nps, TW1], BF16)
    tbl1_full = nc.dram_tensor("tbl1_full", [N, TW1], BF16, addr_space="Shared")
    tbl2_slice = nc.dram_tensor("tbl2_slice", [nps, TW2], BF16)
    tbl2_full = nc.dram_tensor("tbl2_full", [N, TW2], BF16, addr_space="Shared")
    ar_in = nc.dram_tensor("ar_in", [h2], F32)
    ar_out = nc.dram_tensor("ar_out", [h2], F32, addr_space="Shared")

    nb_of = lambda b: min(P, nps - b * P)

    with tile.TileContext(nc) as tc:
        with tc.tile_pool(name="pers", bufs=1) as pers:
            ident = pers.tile([P, P], F32)
            make_identity(nc, ident[:])
            iota_i = pers.tile([P, P], mybir.dt.int32)
            nc.gpsimd.iota(iota_i[:], pattern=[[1, P]], base=0,
                           channel_multiplier=0)
            iota_bf = pers.tile([P, P], BF16)
            nc.vector.tensor_copy(iota_bf[:], iota_i[:])
            ones_col = pers.tile([P, 1], F32)
            nc.vector.memset(ones_col[:], 1.0)
            ones_row = pers.tile([1, P], F32)
            nc.vector.memset(ones_row[:], 1.0)
            onesb_col = pers.tile([P, 1], BF16)
            nc.vector.memset(onesb_col[:], 1.0)

            w1_sb = pers.tile([d_in, AUG1], F32)
            nc.sync.dma_start(out=w1_sb[:], in_=W1a[:])
            w2_sb = pers.tile([h1, AUG2], F32)
            nc.sync.dma_start(out=w2_sb[:], in_=W2a[:])
            b1_repl = pers.tile([P, h1], F32)
            b2_repl = pers.tile([P, h2], F32)
            dstrel_sb = pers.tile([P, C_tot], BF16)
            nc.sync.dma_start(out=dstrel_sb[:], in_=dstrel_in[:])

            acc_big = pers.tile([P, NBLK * NWMAX], F32, tag="acc")
            adrep_big = pers.tile([P, NBLK * P], BF16, tag="adrep")
            adcol = pers.tile([P, NBLK], F32)
            exp_big = pers.tile([P, NBLK * h2], F32, tag="exp")
            invcs_repl = pers.tile([P, h2], F32)

            with (
                tc.tile_pool(name="ps_init", bufs=2, space="PSUM") as psi,
                tc.tile_pool(name="sb_init", bufs=2) as sbi,
            ):
                b1row = sbi.tile([1, h1], F32)
                nc.sync.dma_start(out=b1row[:], in_=b1_in[:])
                b2row = sbi.tile([1, h2], F32)
                nc.sync.dma_start(out=b2row[:], in_=b2_in[:])
                pt = psi.tile([P, h1], F32)
                nc.tensor.matmul(pt[:], lhsT=ones_row[:], rhs=b1row[:],
                                 start=True, stop=True)
                nc.vector.tensor_copy(b1_repl[:], pt[:])
                pt2 = psi.tile([P, h2], F32)
                nc.tensor.matmul(pt2[:], lhsT=ones_row[:], rhs=b2row[:],
                                 start=True, stop=True)
                nc.vector.tensor_copy(b2_repl[:], pt2[:])

            # ---------------- phase 0: layer-1 table ----------------
            with (
                tc.tile_pool(name="p0_ps", bufs=3, space="PSUM") as psp,
                tc.tile_pool(name="p0_sb", bufs=3) as sbp,
            ):
                for t in range(NBLK):
                    nb = nb_of(t)
                    xt = sbp.tile([P, d_in], F32, tag="xin")
                    nc.sync.dma_start(out=xt[:nb, :],
                                      in_=x_in[t * P:t * P + nb, :])
                    ptr = psp.tile([P, P], F32, tag="tr")
                    nc.tensor.transpose(ptr[:d_in, :nb], xt[:nb, :],
                                        ident[:nb, :nb])
                    xT = sbp.tile([P, P], F32, tag="xT")
                    nc.vector.tensor_copy(xT[:d_in, :nb], ptr[:d_in, :nb])
                    pmm = psp.tile([P, AUG1], F32, tag="mm")
                    nc.tensor.matmul(pmm[:nb, :], lhsT=xT[:d_in, :nb],
                                     rhs=w1_sb[:], start=True, stop=True)
                    tb = sbp.tile([P, TW1], BF16, tag="tb")
                    nc.vector.memset(tb[:nb, h1 + 2:TW1], 0)
                    nc.vector.tensor_copy(tb[:nb, 0:h1 + 1], pmm[:nb, 0:h1 + 1])
                    nc.vector.tensor_copy(tb[:nb, h1 + 1:h1 + 2], onesb_col[:nb])
                    nc.vector.tensor_copy(adcol[:nb, t:t + 1],
                                          pmm[:nb, h1 + 1:h1 + 2])
                    nc.sync.dma_start(out=tbl1_slice[t * P:t * P + nb, :],
                                      in_=tb[:nb, :])

            nc.gpsimd.collective_compute(
                "AllGather", mybir.AluOpType.bypass, replica_groups=[core_ids],
                ins=[tbl1_slice[:]], outs=[tbl1_full[:]],
            )

            # ---------------- shared edge phase ----------------
            def build_adst_repl():
                with (
                    tc.tile_pool(name="ad_ps", bufs=1, space="PSUM") as adp,
                    tc.tile_pool(name="ad_ps1", bufs=2, space="PSUM") as adp1,
                    tc.tile_pool(name="ad_sb", bufs=4) as ads,
                ):
                    for g0 in range(0, NBLK, 4):
                        gn = min(4, NBLK - g0)
                        prep = adp.tile([P, 4 * P], F32, tag="rep")
                        for i in range(gn):
                            b = g0 + i
                            ptr = adp1.tile([1, P], F32, tag="tr")
                            nc.tensor.transpose(ptr[:1, :P], adcol[:, b:b + 1],
                                                ident[:])
                            row = ads.tile([1, P], F32, tag="row")
                            nc.vector.tensor_copy(row[:], ptr[:1, :])
                            nc.tensor.matmul(prep[:, i * P:(i + 1) * P],
                                             lhsT=ones_row[:], rhs=row[:],
                                             start=True, stop=True)
                        nc.vector.tensor_copy(
                            adrep_big[:, g0 * P:(g0 + gn) * P],
                            prep[:, 0:gn * P])

            def edge_phase(tbl_full, tw, nw, asrc_col, finalize_fn, finp, fins):
                build_adst_repl()
                with (
                    tc.tile_pool(name="eg_ps", bufs=4, space="PSUM") as egp,
                    tc.tile_pool(name="eg_g", bufs=2) as gp,
                    tc.tile_pool(name="eg_oh", bufs=2) as ohp,
                    tc.tile_pool(name="eg_pr", bufs=1) as prp,
                    tc.tile_pool(name="eg_s", bufs=3) as sp,
                ):
                    psum_live = {}
                    for w in sched["windows"]:
                        k, c0, cw = w["k"], w["c0"], w["cw"]
                        rows_k = sched["bucket_rows"][k]
                        idx_t = sp.tile([P, WCOLS * 8], I16, tag="idx")
                        nc.sync.dma_start(out=idx_t[:, 0:cw * 8],
                                          in_=gidx_in[:, c0 * 8:(c0 + cw) * 8])
                        G = gp.tile([P, WCOLS, tw], BF16, tag="G")
                        nc.gpsimd.dma_gather(
                            out_ap=G[:, 0:cw, :],
                            in_ap=tbl_full[k * BK:k * BK + rows_k, :],
                            idxs_ap=idx_t[:, 0:cw * 8],
                            num_idxs=cw * P, num_idxs_reg=cw * P,
                            elem_size=tw, single_packet=False,
                        )
                        oh = ohp.tile([P, WCOLS, P], BF16, tag="oh")
                        rel_bc = dstrel_sb[:, c0:c0 + cw].rearrange(
                            "p (c o) -> p c o", o=1).to_broadcast([P, cw, P])
                        iota_bc = iota_bf[:].rearrange(
                            "p (o d) -> p o d", o=1).to_broadcast([P, cw, P])
                        nc.vector.tensor_tensor(
                            out=oh[:, 0:cw, :], in0=iota_bc, in1=rel_bc,
                            op=mybir.AluOpType.is_equal)
                        prod = prp.tile([P, WCOLS, P], BF16, tag="prod")
                        for s in w["segs"]:
                            b, lo, hi = s["b"], s["lo"], s["hi"]
                            ad_bc = adrep_big[:, b * P:(b + 1) * P].rearrange(
                                "p (o d) -> p o d", o=1).to_broadcast(
                                    [P, hi - lo, P])
                            nc.vector.tensor_tensor(
                                out=prod[:, lo:hi, :], in0=oh[:, lo:hi, :],
                                in1=ad_bc, op=mybir.AluOpType.mult)
                        adst_e = sp.tile([P, WCOLS], F32, tag="adst")
                        nc.vector.tensor_reduce(
                            out=adst_e[:, 0:cw], in_=prod[:, 0:cw, :],
                            axis=mybir.AxisListType.X, op=mybir.AluOpType.add)
                        asrc_f = sp.tile([P, WCOLS], F32, tag="asrc")
                        nc.vector.tensor_copy(asrc_f[:, 0:cw],
                                              G[:, 0:cw, asrc_col])
                        tt = sp.tile([P, WCOLS], F32, tag="tt")
                        nc.vector.tensor_add(tt[:, 0:cw], asrc_f[:, 0:cw],
                                             adst_e[:, 0:cw])
                        t2 = sp.tile([P, WCOLS], F32, tag="t2")
                        nc.vector.tensor_scalar_mul(t2[:, 0:cw], tt[:, 0:cw],
                                                    NEG_SLOPE_GAT)
                        nc.vector.tensor_tensor(out=tt[:, 0:cw], in0=tt[:, 0:cw],
                                                in1=t2[:, 0:cw],
                                                op=mybir.AluOpType.max)
                        wbf = sp.tile([P, WCOLS], BF16, tag="wbf")
                        nc.scalar.activation(wbf[:, 0:cw], tt[:, 0:cw],
                                             mybir.ActivationFunctionType.Exp)
                        Gw = gp.tile([P, WCOLS, nw], BF16, tag="Gw")
                        w_bc = wbf[:, 0:cw].rearrange(
                            "p (c o) -> p c o", o=1).to_broadcast([P, cw, nw])
                        nc.vector.tensor_tensor(
                            out=Gw[:, 0:cw, :], in0=G[:, 0:cw, 0:nw], in1=w_bc,
                            op=mybir.AluOpType.mult)

                        for s in w["segs"]:
                            b, lo, hi = s["b"], s["lo"], s["hi"]
                            fp = int(sched["first_pass"][b])
                            lp = int(sched["last_pass"][b])
                            if s["first"]:
                                pb = egp.tile([P, nw], F32, tag="agg")
                                psum_live[b] = pb
                                if k > fp:
                                    nc.tensor.matmul(
                                        pb[:], lhsT=ident[:],
                                        rhs=acc_big[:, b * NWMAX:b * NWMAX + nw],
                                        start=True, stop=False,
                                        skip_group_check=True)
                            pb = psum_live[b]
                            for j in range(lo, hi):
                                first_mm = (k == fp and s["first"] and j == lo)
                                last_mm = (s["last"] and j == hi - 1)
                                nc.tensor.matmul(
                                    pb[:], lhsT=oh[:, j, :], rhs=Gw[:, j, :],
                                    start=first_mm, stop=last_mm,
                                    skip_group_check=True)
                            if s["last"]:
                                del psum_live[b]
                                if k < lp:
                                    nc.scalar.copy(
                                        acc_big[:, b * NWMAX:b * NWMAX + nw],
                                        pb[:])
                                else:
                                    finalize_fn(b, pb, finp, fins)

            # ---- layer-1 finalize: h tile -> layer-2 table rows ----
            def fin1(b, pb, finp, fins):
                nb = nb_of(b)
                inv = fins.tile([P, 1], F32, tag="inv")
                nc.vector.reciprocal(inv[:nb], pb[:nb, NW1 - 1:NW1])
                y0 = fins.tile([P, h1], F32, tag="y0")
                nc.scalar.activation(y0[:nb, :], pb[:nb, 0:h1],
                                     mybir.ActivationFunctionType.Identity,
                                     scale=inv[:nb])
                nc.vector.tensor_add(y0[:nb, :], y0[:nb, :], b1_repl[:nb, :])
                y2 = fins.tile([P, h1], F32, tag="y2")
                nc.vector.tensor_scalar_mul(y2[:nb, :], y0[:nb, :],
                                            NEG_SLOPE_ACT)
                nc.vector.tensor_tensor(out=y0[:nb, :], in0=y0[:nb, :],
                                        in1=y2[:nb, :], op=mybir.AluOpType.max)
                ptr = finp.tile([P, P], F32, tag="tr1")
                nc.tensor.transpose(ptr[:h1, :nb], y0[:nb, :], ident[:nb, :nb])
                hT = fins.tile([P, P], F32, tag="hT")
                nc.vector.tensor_copy(hT[:h1, :nb], ptr[:h1, :nb])
                pmm = finp.tile([P, AUG2], F32, tag="mm1")
                nc.tensor.matmul(pmm[:nb, :], lhsT=hT[:h1, :nb], rhs=w2_sb[:],
                                 start=True, stop=True)
                tb = fins.tile([P, TW2], BF16, tag="tb2")
                nc.vector.memset(tb[:nb, h2 + 2:TW2], 0)
                nc.vector.tensor_copy(tb[:nb, 0:h2 + 1], pmm[:nb, 0:h2 + 1])
                nc.vector.tensor_copy(tb[:nb, h2 + 1:h2 + 2], onesb_col[:nb])
                nc.vector.tensor_copy(adcol[:nb, b:b + 1],
                                      pmm[:nb, h2 + 1:h2 + 2])
                nc.sync.dma_start(out=tbl2_slice[b * P:b * P + nb, :],
                                  in_=tb[:nb, :])

            with (
                tc.tile_pool(name="f1_ps", bufs=1, space="PSUM") as finp1,
                tc.tile_pool(name="f1_sb", bufs=2) as fins1,
            ):
                edge_phase(tbl1_full, TW1, NW1, h1, fin1, finp1, fins1)

            nc.gpsimd.collective_compute(
                "AllGather", mybir.AluOpType.bypass, replica_groups=[core_ids],
                ins=[tbl2_slice[:]], outs=[tbl2_full[:]],
            )

            # ---- layer-2 finalize: exp tiles + column sums ----
            cs_state = {"n": 0}

            def fin2(b, pb, finp, fins):
                nb = nb_of(b)
                inv = fins.tile([P, 1], F32, tag="inv2")
                nc.vector.reciprocal(inv[:nb], pb[:nb, NW2 - 1:NW2])
                y0 = fins.tile([P, h2], F32, tag="y02")
                nc.scalar.activation(y0[:nb, :], pb[:nb, 0:h2],
                                     mybir.ActivationFunctionType.Identity,
                                     scale=inv[:nb])
                nc.vector.tensor_add(y0[:nb, :], y0[:nb, :], b2_repl[:nb, :])
                esl = exp_big[:nb, b * h2:(b + 1) * h2]
                nc.scalar.activation(esl, y0[:nb, :],
                                     mybir.ActivationFunctionType.Exp)
                if cs_state["n"] == 0:
                    cs_tile = cs_state["pool"].tile([h2, 1], F32, tag="cs")
                    cs_state["cs"] = cs_tile
                cs_state["n"] += 1
                nc.tensor.matmul(cs_state["cs"][:], lhsT=esl,
                                 rhs=ones_col[:nb],
                                 start=(cs_state["n"] == 1),
                                 stop=(cs_state["n"] == NBLK),
                                 skip_group_check=True)

            with (
                tc.tile_pool(name="cs_ps", bufs=1, space="PSUM") as csp,
                tc.tile_pool(name="f2_ps", bufs=2, space="PSUM") as finp2,
                tc.tile_pool(name="f2_sb", bufs=2) as fins2,
            ):
                cs_state["pool"] = csp
                edge_phase(tbl2_full, TW2, NW2, h2, fin2, finp2, fins2)

                # ---- global softmax over axis 0 ----
                with (
                    tc.tile_pool(name="sm_ps", bufs=2, space="PSUM") as smp,
                    tc.tile_pool(name="sm_sb", bufs=2) as sms,
                ):
                    cs_sb = sms.tile([h2, 1], F32, tag="cs_sb")
                    nc.vector.tensor_copy(cs_sb[:], cs_state["cs"][:])
                    nc.sync.dma_start(out=ar_in[:], in_=cs_sb[:])
                    nc.gpsimd.collective_compute(
                        "AllReduce", mybir.AluOpType.add,
                        replica_groups=[core_ids],
                        ins=[ar_in[:]], outs=[ar_out[:]],
                    )
                    cst = sms.tile([h2, 1], F32, tag="cst")
                    nc.sync.dma_start(out=cst[:], in_=ar_out[:])
                    nc.vector.reciprocal(cst[:], cst[:])
                    ptr = smp.tile([1, P], F32, tag="trc")
                    nc.tensor.transpose(ptr[:1, :h2], cst[:], ident[:h2, :h2])
                    row = sms.tile([1, h2], F32, tag="rowc")
                    nc.vector.tensor_copy(row[:], ptr[:1, :h2])
                    prep = smp.tile([P, h2], F32, tag="repc")
                    nc.tensor.matmul(prep[:], lhsT=ones_row[:], rhs=row[:],
                                     start=True, stop=True)
                    nc.vector.tensor_copy(invcs_repl[:], prep[:])
                    for b in range(NBLK):
                        nb = nb_of(b)
                        ot = sms.tile([P, h2], F32, tag="ot")
                        nc.vector.tensor_mul(ot[:nb, :],
                                             exp_big[:nb, b * h2:(b + 1) * h2],
                                             invcs_repl[:nb, :])
                        nc.sync.dma_start(out=out_ext[b * P:b * P + nb, :],
                                          in_=ot[:nb, :])

    return nc


# ----------------------------------------------------------------------------
# Public entry point
# ----------------------------------------------------------------------------

_LAST = {}


def _run(x, edge_index, W1, att_src1, att_dst1, b1, W2, att_src2, att_dst2, b2,
         n_cores=8, BK=25000):
    x = np.asarray(x, dtype=np.float32)
    N, d_in = x.shape
    h1 = np.asarray(W1).shape[1]
    h2 = np.asarray(W2).shape[1]
    ei = np.asarray(edge_index)
    loops = np.arange(N, dtype=np.int64)
    src = np.concatenate([ei[0], loops]).astype(np.int64)
    dst = np.concatenate([ei[1], loops]).astype(np.int64)

    sched = _build_schedule(src, dst, N, n_cores, BK)
    gidx_all, dstrel_all = _build_edge_arrays(src, dst, sched)
    nc = _build_program(sched, d_in, h1, h2)
    nc.finalize()

    W1 = np.asarray(W1, np.float32)
    W2 = np.asarray(W2, np.float32)
    AUG1, AUG2 = d_in + 4, h1 + 8
    W1a = np.zeros((d_in, AUG1), np.float32)
    W1a[:, :h1] = W1
    W1a[:, h1] = W1 @ np.asarray(att_src1, np.float32)
    W1a[:, h1 + 1] = W1 @ np.asarray(att_dst1, np.float32)
    W2a = np.zeros((h1, AUG2), np.float32)
    W2a[:, :h2] = W2
    W2a[:, h2] = W2 @ np.asarray(att_src2, np.float32)
    W2a[:, h2 + 1] = W2 @ np.asarray(att_dst2, np.float32)

    nps = sched["nps"]
    in_maps = []
    for c in range(n_cores):
        in_maps.append({
            "x_slice": np.ascontiguousarray(x[c * nps:(c + 1) * nps]),
            "W1_aug": W1a, "W2_aug": W2a,
            "b1r": np.asarray(b1, np.float32).reshape(1, -1),
            "b2r": np.asarray(b2, np.float32).reshape(1, -1),
            "gidx": gidx_all[c], "dstrel": dstrel_all[c],
        })
    _LAST.update(nc=nc, in_maps=in_maps, n_cores=n_cores)
    res = run_bass_kernel_spmd(nc, in_maps, list(range(n_cores)))
    out = np.concatenate([res.results[c]["out_slice"] for c in range(n_cores)],
                         axis=0)
    return out


def kernel(x, edge_index, W1, att_src1, att_dst1, b1, W2, att_src2, att_dst2,
           b2):
    return _run(x, edge_index, W1, att_src1, att_dst1, b1,
                W2, att_src2, att_dst2, b2, n_cores=8, BK=25000)
